# revision 16
# baseline (speedup 1.0000x reference)
"""Trainium2 Bass kernel for single-head causal attention (v9).

Math as v7 (transposed softmax S^T, WQ folded into the key side,
interleaved-key causal skip, host flash-combine); V-first schedule:

* The V projection runs FIRST: it needs only ~4MB of input DMA (bf16
  xv + wv), so compute starts early while the heavy K-path weights
  (wk, wq, xk — ~9.5MB fp32) stream in its shadow.  K then runs fully
  fed, KQ after it, then the attention groups.
* PE warm-up block at t=0 covers the ~8us multi-core start barrier +
  the first V input DMA.
* All streamed tensors are host-repacked so every DMA slice is
  contiguous per partition row (4-16KB lines, one descriptor each).
* Trigger queues: scalar = wv, wk, mask, wq(3 early + 5 streamed
  in-loop), l rows + ohat writes during attention.  sync = xv chunks,
  xk chunks (3 early + 1 in-loop), then xq group prefetches.
  A dma trigger only fires when the issuing engine's stream reaches
  it, and a trigger with a pool-rotation WAR blocks that engine -- so
  triggers are placed to fire exactly when wanted and rotation-gated
  triggers are never emitted ahead of the instructions they wait on.
* One PSUM pool spans V/K/KQ (tag rotation, no boundary syncs).
* Attention software-pipelined ascending: S0 S1 l0 O0 S2 l1 O1 S3 l2
  O2 l3 O3; exp/DVE latency hides under the next group's matmuls; the
  big last group hides its own prefetch and the output drain.
* Causal mask is group-invariant: one [128, 512] bf16 tile per core.
* V path bf16 (xv, wv, ohat); score path fp32r (bf16 fails the gate).
  1/sqrt(d) folded into the exp scale; softmax denominator via DVE
  accumulation + one ones-matmul per group.

Outputs: ohat [2048, 1024] bf16 (unnormalized), l [1, 2048] f32;
host combines out = (ohat0 + ohat1) / (l0 + l1).
"""

import ml_dtypes
import numpy as np

import concourse.bass as bass
from concourse import bacc
import concourse.mybir as mybir
import concourse.tile as tile
from concourse.bass_utils import run_bass_kernel_spmd

P = 128
B, S, DIN, DOUT = 4, 2048, 1024, 1024
KSH = S // 2        # key rows per core
KO = DIN // P       # 8 contraction sub-tiles
DO = DOUT // P      # 8 dout sub-tiles
NT = KSH // P       # 8 key tiles per core
QG = 512            # query-group width (psum free dim)
NG = S // QG        # 4 query groups
H = QG // 2
KCH = 256           # xk stream chunk width
NKC = KSH // KCH    # 4 xk chunks
VCH = 256           # xv stream chunk width
NVC = KSH // VCH    # 4 xv chunks
F32 = mybir.dt.float32
F32R = mybir.dt.float32r
BF16 = mybir.dt.bfloat16
SCALE = 1.0 / float(np.sqrt(DOUT))
NEG = -1.0e9
NWARM = 28          # PE warm-up matmuls
EXP = mybir.ActivationFunctionType.Exp

_NC_CACHE = {}


def _emit_score_chunk(nc, psS, mfull_sb, kqt, xq_g, slab, zeros_sb, g, kt):
    """Scores+exp for one (group, k-tile) chunk of S^T.

    kqt is unscaled; 1/sqrt(d) is folded into the exp's scale (the
    additive mask scales along, -1e9/32 still floors the exp).
    kt == 2g+1: first 256 query columns fully masked -> zero-fill,
    compute the second half only.  kt == 2g: diagonal, additive mask.
    """
    if kt == 2 * g + 1:
        ps = psS.tile([P, QG], F32, name="score_ps")
        ph = ps[:, H:]
        for io in range(KO):
            nc.tensor.matmul(
                ph, lhsT=kqt[:, io, kt * P : (kt + 1) * P],
                rhs=xq_g[:, io, H:],
                start=(io == 0), stop=(io == KO - 1),
            )
        nc.vector.tensor_copy(slab[:, kt, :H], zeros_sb[:, :H])
        nc.vector.tensor_tensor(
            slab[:, kt, H:], ph, mfull_sb[:, :H], mybir.AluOpType.add
        )
        nc.scalar.activation(slab[:, kt, H:], slab[:, kt, H:], EXP, scale=SCALE)
        return
    ps = psS.tile([P, QG], F32, name="score_ps")
    for io in range(KO):
        nc.tensor.matmul(
            ps, lhsT=kqt[:, io, kt * P : (kt + 1) * P], rhs=xq_g[:, io, :],
            start=(io == 0), stop=(io == KO - 1),
        )
    if kt == 2 * g:
        nc.vector.tensor_tensor(
            slab[:, kt, :], ps, mfull_sb, mybir.AluOpType.add
        )
        nc.scalar.activation(slab[:, kt, :], slab[:, kt, :], EXP, scale=SCALE)
    else:
        nc.scalar.activation(slab[:, kt, :], ps, EXP, scale=SCALE)


def _build_bass():
    nc = bacc.Bacc()
    # host-repacked layouts: every DMA slice is contiguous per partition row
    xq4 = nc.declare_dram_parameter("xq4", [P, NG, KO, QG], F32R, isOutput=False)
    xk4 = nc.declare_dram_parameter("xk4", [P, NKC, KO, KCH], F32R, isOutput=False)
    xv4 = nc.declare_dram_parameter("xv4", [P, NVC, KO, VCH], BF16, isOutput=False)
    wq4 = nc.declare_dram_parameter("wq4", [P, KO, DO, P], F32R, isOutput=False)
    wk4 = nc.declare_dram_parameter("wk4", [P, DO, KO, P], F32R, isOutput=False)
    wv4 = nc.declare_dram_parameter("wv4", [P, 2, KO, 512], BF16, isOutput=False)
    mfull = nc.declare_dram_parameter("mfull", [P, QG], BF16, isOutput=False)
    ohat = nc.declare_dram_parameter("ohat", [S, DOUT], BF16, isOutput=True)
    l_out = nc.declare_dram_parameter("l", [1, S], F32, isOutput=True)

    with tile.TileContext(nc) as tc:
        # ---- constants
        cpool_cm = tc.tile_pool(name="const", bufs=1)
        cpool = cpool_cm.__enter__()
        z32 = cpool.tile([P, QG], F32, name="z32")
        nc.vector.memset(z32, 0.0)
        zeros_sb = cpool.tile([P, QG], F32R, name="zeros")
        nc.vector.tensor_copy(zeros_sb, z32)
        ones32 = cpool.tile([P, 1], F32, name="ones32")
        nc.vector.memset(ones32, 1.0)
        ones_sb = cpool.tile([P, 1], F32R, name="ones")
        nc.vector.tensor_copy(ones_sb, ones32)
        mfull_sb = cpool.tile([P, QG], BF16, name="mfull_sb")

        # ---- PE warm-up through the start barrier + first input DMA
        with tc.tile_pool(name="ps_warm", bufs=1, space="PSUM") as psW:
            pw = psW.tile([P, QG], F32, name="warm_ps")
            for _ in range(NWARM):
                nc.tensor.matmul(
                    pw, lhsT=zeros_sb[:, :P], rhs=zeros_sb,
                    start=True, stop=True,
                )

        with tc.tile_pool(name="persist", bufs=1) as persist:
            v_sb = persist.tile([P, NT, DOUT], F32R, name="v")
            kqt_sb = persist.tile([P, KO, KSH], F32R, name="kqt")

            # created first so it can outlive the K-path pools (LIFO)
            xqpool_cm = tc.tile_pool(name="xq_pool", bufs=2)
            xqpool = xqpool_cm.__enter__()

            wkpool_cm = tc.tile_pool(name="wk_pool", bufs=1)
            wkpool = wkpool_cm.__enter__()
            wk_sb = wkpool.tile([P, KO, DOUT], F32R, name="wk_sb")

            wqpool_cm = tc.tile_pool(name="wq_pool", bufs=3)
            wqpool = wqpool_cm.__enter__()

            xkpool_cm = tc.tile_pool(name="xk_pool", bufs=3)
            xkpool = xkpool_cm.__enter__()

            wvpool_cm = tc.tile_pool(name="wv_pool", bufs=1)
            wvpool = wvpool_cm.__enter__()
            wv_sb = wvpool.tile([P, KO, DOUT], BF16, name="wv_sb")

            xvpool_cm = tc.tile_pool(name="xv_pool", bufs=4)
            xvpool = xvpool_cm.__enter__()

            # ---- early triggers.
            # scalar: wv (V needs it first), then wk, mask, first wq slices.
            for dh in range(2):
                nc.scalar.dma_start(
                    out=wv_sb[:, :, dh * 512 : (dh + 1) * 512],
                    in_=wv4[:, dh, :, :],
                )
            for s_ in range(DO):
                nc.scalar.dma_start(
                    out=wk_sb[:, :, s_ * P : (s_ + 1) * P], in_=wk4[:, s_, :, :]
                )
            nc.scalar.dma_start(out=mfull_sb, in_=mfull[:, :])
            wq_tiles = {}
            for it in range(3):
                wq_tiles[it] = wqpool.tile([P, DO, P], F32R, name="wq_sl")
                nc.scalar.dma_start(out=wq_tiles[it][:, :, :], in_=wq4[:, it, :, :])
            # sync: all xv chunks (bufs=4, no rotation waits).
            xv_tiles = []
            for c in range(NVC):
                xv_c = xvpool.tile([P, KO, VCH], BF16, name="xv_chunk")
                nc.sync.dma_start(out=xv_c[:, :, :], in_=xv4[:, c, :, :])
                xv_tiles.append(xv_c)

            # one PSUM pool spans V/K/KQ: tag rotation, no boundary syncs
            pspool_cm = tc.tile_pool(name="ps_main", bufs=4, space="PSUM")
            pspool = pspool_cm.__enter__()

            # first 3 xk chunk triggers (behind xv on sync; the 4th chunk
            # is rotation-gated so it is emitted in-loop).
            xk_tiles = {}
            for c in range(3):
                xk_tiles[c] = xkpool.tile([P, KO, KCH], F32R, name="xk_chunk")
                nc.sync.dma_start(out=xk_tiles[c][:, :, :], in_=xk4[:, c, :, :])

            # ---- Phase V: V = Xv @ WV (bf16 in, fp32 accum)
            for c in range(NVC):
                for t in range(VCH // P):
                    for dh in range(DOUT // 512):
                        ps = pspool.tile([P, QG], F32, name="mm_ps")
                        for k in range(KO):
                            nc.tensor.matmul(
                                ps,
                                lhsT=xv_tiles[c][:, k, t * P : (t + 1) * P],
                                rhs=wv_sb[:, k, dh * 512 : (dh + 1) * 512],
                                start=(k == 0),
                                stop=(k == KO - 1),
                            )
                        nc.vector.tensor_copy(
                            v_sb[
                                :,
                                c * (VCH // P) + t,
                                dh * 512 : (dh + 1) * 512,
                            ],
                            ps,
                        )

            xvpool_cm.__exit__(None, None, None)
            wvpool_cm.__exit__(None, None, None)

            # kt only exists K -> KQ; allocated in the slot xv/wv vacated
            ktpool_cm = tc.tile_pool(name="kt_pool", bufs=1)
            ktpool = ktpool_cm.__enter__()
            kt_sb = ktpool.tile([P, DO, KSH], F32R, name="kt")

            # ---- Phase K: K^T = WK^T @ Xk^T
            for c in range(NKC):
                if c + 3 < NKC:
                    xk_tiles[c + 3] = xkpool.tile([P, KO, KCH], F32R, name="xk_chunk")
                    nc.sync.dma_start(
                        out=xk_tiles[c + 3][:, :, :], in_=xk4[:, c + 3, :, :]
                    )
                x_sb = xk_tiles[c]
                for o in range(DO):
                    ps = pspool.tile([P, QG], F32, name="mm_ps")
                    psn = ps[:, :KCH]
                    for k in range(KO):
                        nc.tensor.matmul(
                            psn,
                            lhsT=wk_sb[:, k, o * P : (o + 1) * P],
                            rhs=x_sb[:, k, :],
                            start=(k == 0),
                            stop=(k == KO - 1),
                        )
                    nc.vector.tensor_copy(
                        kt_sb[:, o, c * KCH : (c + 1) * KCH], psn
                    )

            # query-group prefetches for the first two attention groups
            # (sync stream reaches these at K retire; 2MB each).
            xq_tiles = {}
            for g in (0, 1):
                xq_tiles[g] = xqpool.tile([P, KO, QG], F32R, name="xq_group")
                nc.sync.dma_start(out=xq_tiles[g][:, :, :], in_=xq4[:, g, :, :])

            # ---- Phase KQ: KQ^T = WQ @ K^T (unscaled)
            for it in range(KO):
                wq_t = wq_tiles[it]
                for kc in range(KSH // 512):
                    ps = pspool.tile([P, QG], F32, name="mm_ps")
                    for do in range(DO):
                        nc.tensor.matmul(
                            ps,
                            lhsT=wq_t[:, do, :],
                            rhs=kt_sb[:, do, kc * 512 : (kc + 1) * 512],
                            start=(do == 0),
                            stop=(do == DO - 1),
                        )
                    nc.vector.tensor_copy(
                        kqt_sb[:, it, kc * 512 : (kc + 1) * 512], ps
                    )
                if it + 3 < KO:
                    wq_tiles[it + 3] = wqpool.tile([P, DO, P], F32R, name="wq_sl")
                    nc.scalar.dma_start(
                        out=wq_tiles[it + 3][:, :, :], in_=wq4[:, it + 3, :, :]
                    )

            pspool_cm.__exit__(None, None, None)

            # ---- attention pools (kt/xk/wq/wk freed first, LIFO)
            ktpool_cm.__exit__(None, None, None)
            xkpool_cm.__exit__(None, None, None)
            wqpool_cm.__exit__(None, None, None)
            wkpool_cm.__exit__(None, None, None)

            slabpool_cm = tc.tile_pool(name="slab", bufs=2)
            slabpool = slabpool_cm.__enter__()
            accpool_cm = tc.tile_pool(name="l_acc", bufs=2)
            accpool = accpool_cm.__enter__()
            lpool_cm = tc.tile_pool(name="l_row", bufs=2)
            lpool = lpool_cm.__enter__()
            opool_cm = tc.tile_pool(name="o_out", bufs=3)
            opool = opool_cm.__enter__()

            # ---- Attention, software-pipelined ascending groups:
            # S0 S1 l0 O0 S2 l1 O1 S3 l2 O2 l3 O3
            with (
                tc.tile_pool(name="ps_s", bufs=4, space="PSUM") as psS,
                tc.tile_pool(name="ps_l", bufs=1, space="PSUM") as psL,
                tc.tile_pool(name="ps_o", bufs=3, space="PSUM") as psO,
            ):
                slabs = {}

                def emit_scores(g):
                    lim = min(NT, 2 * g + 2)
                    slab = slabpool.tile([P, NT, QG], F32R, name="expT")
                    slabs[g] = slab
                    for kt in range(lim):
                        _emit_score_chunk(
                            nc, psS, mfull_sb, kqt_sb, xq_tiles[g],
                            slab, zeros_sb, g, kt,
                        )
                    if g + 2 < NG:
                        nxt = xqpool.tile([P, KO, QG], F32R, name="xq_group")
                        xq_tiles[g + 2] = nxt
                        nc.sync.dma_start(
                            out=nxt[:, :, :], in_=xq4[:, g + 2, :, :]
                        )

                def emit_l(g):
                    lim = min(NT, 2 * g + 2)
                    slab = slabs[g]
                    acc = accpool.tile([P, QG], F32R, name="acc")
                    nc.vector.tensor_tensor(
                        acc, slab[:, 0, :], slab[:, 1, :], mybir.AluOpType.add
                    )
                    for kt in range(2, lim):
                        nc.vector.tensor_tensor(
                            acc, acc, slab[:, kt, :], mybir.AluOpType.add
                        )
                    ps_l = psL.tile([1, QG], F32, name="l_ps")
                    nc.tensor.matmul(
                        ps_l, lhsT=ones_sb, rhs=acc, start=True, stop=True
                    )
                    l_t = lpool.tile([1, QG], F32, name="l_t")
                    nc.vector.tensor_copy(l_t, ps_l)
                    nc.scalar.dma_start(
                        out=l_out[:, g * QG : (g + 1) * QG], in_=l_t
                    )

                def emit_out(g):
                    lim = min(NT, 2 * g + 2)
                    slab = slabs[g]
                    for t in range(QG // P):
                        kts = (
                            list(range(lim - 1)) if t < 2 else list(range(lim))
                        )
                        q0 = g * QG + t * P
                        last_tile = g == NG - 1 and t == QG // P - 1
                        o_sb = opool.tile([P, DOUT], BF16, name="attn_out")
                        for dh in range(DOUT // 512):
                            ps = psO.tile([P, 512], F32, name="out_ps")
                            for kt in kts:
                                nc.tensor.matmul(
                                    ps,
                                    lhsT=slab[:, kt, t * P : (t + 1) * P],
                                    rhs=v_sb[:, kt, dh * 512 : (dh + 1) * 512],
                                    start=(kt == kts[0]),
                                    stop=(kt == kts[-1]),
                                )
                            nc.scalar.copy(
                                o_sb[:, dh * 512 : (dh + 1) * 512], ps
                            )
                            if last_tile:
                                nc.scalar.dma_start(
                                    out=ohat[
                                        q0 : q0 + P, dh * 512 : (dh + 1) * 512
                                    ],
                                    in_=o_sb[:, dh * 512 : (dh + 1) * 512],
                                )
                        if not last_tile:
                            nc.scalar.dma_start(
                                out=ohat[q0 : q0 + P, :], in_=o_sb
                            )

                emit_scores(0)
                emit_scores(1)
                emit_l(0)
                emit_out(0)
                emit_scores(2)
                emit_l(1)
                emit_out(1)
                emit_scores(3)
                emit_l(2)
                emit_out(2)
                emit_l(3)
                emit_out(3)

            opool_cm.__exit__(None, None, None)
            lpool_cm.__exit__(None, None, None)
            accpool_cm.__exit__(None, None, None)
            slabpool_cm.__exit__(None, None, None)
            xqpool_cm.__exit__(None, None, None)
        cpool_cm.__exit__(None, None, None)
    nc.finalize()
    return nc


def _get_nc():
    if "nc" not in _NC_CACHE:
        _NC_CACHE["nc"] = _build_bass()
    return _NC_CACHE["nc"]


def _key_index(hk):
    """Global key rows owned by core hk: interleaved 128-row blocks."""
    blocks = np.arange(hk, S // P, 2)
    return (blocks[:, None] * P + np.arange(P)[None, :]).reshape(-1)


def _mask_full(hk):
    """Group-invariant additive causal mask for the diagonal chunks.

    Chunk kt == 2g covers global keys (4g+hk)*128 + p against queries
    512g + j: causal k > q reduces to hk*128 + p > j for every g; chunk
    kt == 2g+1's live half reduces to this tile's first 256 columns.
    """
    p = np.arange(P)[:, None]
    j = np.arange(QG)[None, :]
    m = np.where(hk * P + p > j, np.float32(NEG), np.float32(0.0))
    return np.ascontiguousarray(m.astype(ml_dtypes.bfloat16))


def kernel(
    inputs_for_keys,
    inputs_for_values,
    inputs_for_queries,
    WK,
    WV,
    WQ,
    _trace=False,
):
    xk = np.asarray(inputs_for_keys, dtype=np.float32)
    xv = np.asarray(inputs_for_values, dtype=np.float32)
    xq = np.asarray(inputs_for_queries, dtype=np.float32)
    wk_ = np.asarray(WK, dtype=np.float32)
    wv_ = np.asarray(WV, dtype=np.float32)
    wq_ = np.asarray(WQ, dtype=np.float32)

    # host repack: [128, slice, ...] with contiguous per-partition rows
    wk4 = np.ascontiguousarray(
        wk_.reshape(KO, P, DO, P).transpose(1, 2, 0, 3)
    )
    wq4 = np.ascontiguousarray(
        wq_.T.reshape(DO, P, KO, P).transpose(1, 2, 0, 3)
    )
    wv4 = np.ascontiguousarray(
        wv_.astype(ml_dtypes.bfloat16).reshape(KO, P, 2, 512).transpose(1, 2, 0, 3)
    )

    kidx = {hk: _key_index(hk) for hk in (0, 1)}
    masks = {hk: _mask_full(hk) for hk in (0, 1)}
    xq4b = [
        np.ascontiguousarray(
            xq[b].T.reshape(KO, P, NG, QG).transpose(1, 2, 0, 3)
        )
        for b in range(B)
    ]

    in_maps = []
    for i in range(8):
        b, hk = i // 2, i % 2
        xkT = xk[b][kidx[hk]].T      # [DIN, KSH]
        xvT = xv[b][kidx[hk]].T
        in_maps.append(
            {
                "xq4": xq4b[b],
                "xk4": np.ascontiguousarray(
                    xkT.reshape(KO, P, NKC, KCH).transpose(1, 2, 0, 3)
                ),
                "xv4": np.ascontiguousarray(
                    xvT.astype(ml_dtypes.bfloat16)
                    .reshape(KO, P, NVC, VCH)
                    .transpose(1, 2, 0, 3)
                ),
                "wq4": wq4,
                "wk4": wk4,
                "wv4": wv4,
                "mfull": masks[hk],
            }
        )

    nc = _get_nc()
    res = run_bass_kernel_spmd(nc, in_maps, list(range(8)), trace=_trace)

    out = np.empty((B, S, DOUT), dtype=np.float32)
    for b in range(B):
        r0 = res.results[2 * b]
        r1 = res.results[2 * b + 1]
        den = (r0["l"] + r1["l"]).reshape(S, 1)
        out[b] = (
            r0["ohat"].astype(np.float32) + r1["ohat"].astype(np.float32)
        ) / den
    if _trace:
        return out, res
    return out
